# revision 64
# baseline (speedup 1.0000x reference)
"""GraphWeightedMHA on 8 trn2 cores — head-sharded bf16 Bass/Tile kernel.

Sharding: one attention head per core (tensor parallel); sgconv + final
projection row-sharded. Only the per-head [4096, 64] attention output is
AllGathered (4 chunks issued at qb2/qb4/qb7/end so the collectives and
gather-side transposes overlap attention).

Key scheduling facts this kernel is built around (from HW traces):
- Every DMA kick costs ~0.65us (plain) / ~1.3us (XBAR transpose) of
  in-order sync-engine issue time, and a kick inline-blocks the engine on
  its wait semaphore. So: few fat input DMAs (re-blocked host layouts with
  2-16KB contiguous partition lines), and each AG chunk's gather-side
  transposes are emitted one chunk LATE — behind the next chunk's bounce
  DMA + collective trigger — so the AG chain never serializes on
  transpose kicks waiting for the previous AG.
- DMA kicks must stay off compute engines (scalar-queue kicks interleave
  guard waits with ACT work and can near-deadlock).
- exp split 9 ACT / 7 DVE pairs (ACT 1.10us, DVE 1.22us per [128,1024]
  fp32->bf16 tile; fp32 PSUM reads cap DVE at 1 elem/cycle).
- sgconv runs jt 0..31 in order right behind the last attention block;
  its first ~24us of matmuls hide the final AG + transposes.

All matmuls bf16 (fp32 PSUM accumulation); q/k inputs fp8.
"""
import numpy as np
import ml_dtypes
import concourse.bass as bass
import concourse.bacc as bacc
import concourse.tile as tile
from concourse import mybir
from concourse.bass_utils import run_bass_kernel_spmd

dt = mybir.dt
bf16 = ml_dtypes.bfloat16
fp8 = ml_dtypes.float8_e4m3fn
NC = 8
N, D, H, HD = 4096, 512, 8, 64
RS = N // NC          # 512 output rows per core for sgconv/final proj
NB = D // 128         # 4 blocks of 128 along D
QB = 8                # query blocks of 512
KT = N // 128         # 32 key tiles
NPAIR = KT // 2       # 16 S^T pairs per query block
SCALE = float(1.0 / np.sqrt(np.float32(D)))
Exp = mybir.ActivationFunctionType.Exp
Ident = mybir.ActivationFunctionType.Identity
Mult = mybir.AluOpType.mult
Add = mybir.AluOpType.add

# Schraudolph fast-exp in bf16 bit space: bf16_bits(exp(x)) ~ round(A*x + B)
FEXP_A = float(np.float32(128.0 / np.log(2.0)))
FEXP_B = float(np.float32(127 * 128 - 5.0))
# pairs using the scalar engine's exact exp (rest: DVE Schraudolph fast-exp)
ACT_SET = {0, 2, 4, 6, 8, 10, 12, 14, 15}

# AllGather chunks: (row0, row1, issue_at_qb) — issue_at_qb None = post-loop
AG_CHUNKS = [(0, 1024, 2), (1024, 2048, 4), (2048, 3584, 7), (3584, 4096, None)]

_CACHE: dict = {}


def blk(x):  # [512, M] -> [128, 4, M]  (p, kb, m) with d = kb*128+p
    return np.ascontiguousarray(x.reshape(NB, 128, -1).transpose(1, 0, 2))


def blkq(x):  # [512, N] -> [128, QB, NB, 512]: column-chunk-major for fat DMA
    return np.ascontiguousarray(
        blk(x).reshape(128, NB, QB, 512).transpose(0, 2, 1, 3))


def _build():
    nc = bacc.Bacc("TRN2", target_bir_lowering=False, debug=False, num_devices=NC)

    def din(name, shape, d=dt.bfloat16):
        return nc.dram_tensor(name, shape, d, kind="ExternalInput").ap()

    qT_d = din("qT", [128, QB, NB, 512], dt.float8e4)  # query^T re-blocked (fp8)
    kT_d = din("kT", [128, QB, NB, 512], dt.float8e4)  # key^T re-blocked (fp8)
    vT_d = din("vT", [128, QB, NB, 512])       # value^T re-blocked (bf16)
    wq_d = din("wq", [128, NB, HD], dt.float8e4)  # (64*s*Wq_h)^T blocked
    wk_d = din("wk", [128, NB, HD], dt.float8e4)  # (64*Wk_h)^T blocked
    wv_d = din("wv", [128, NB, HD])            # Wv_h^T blocked (bf16)
    wo_d = din("wo", [128, NB, D])             # Wo^T blocked
    bqk_d = din("bqk", [128, 1], dt.float32)   # [s*bq_h ; bk_h]
    bv_d = din("bv", [1, HD])                  # bv_h row (bf16)
    bo_d = din("bo", [1, D])
    ones_d = din("ones", [128, 128])
    sgT_d = din("sgT", [128, KT, RS])          # sg[rows,:].T pre-blocked, bf16
    out_d = nc.dram_tensor("out", [RS, D], dt.float32, kind="ExternalOutput").ap()

    with tile.TileContext(nc) as tc:
        with tc.tile_pool(name="const", bufs=1) as cp, \
             tc.tile_pool(name="persist", bufs=1) as pp, \
             tc.tile_pool(name="dram", bufs=1, space="DRAM") as dp:
            wq_sb = cp.tile([128, NB, HD], dt.float8e4)
            wk_sb = cp.tile([128, NB, HD], dt.float8e4)
            wv_sb = cp.tile([128, NB, HD], dt.bfloat16)
            wo_sb = cp.tile([128, NB, D], dt.bfloat16)
            bqk_sb = cp.tile([128, 1], dt.float32)
            bv_sb = cp.tile([1, HD], dt.bfloat16)
            bo_sb = cp.tile([1, D], dt.bfloat16)
            ones_sb = cp.tile([128, 128], dt.bfloat16)
            # critical-path weights first; wo/bo (final proj only) last
            for sb_t, d_t in [(wq_sb, wq_d), (wk_sb, wk_d), (wv_sb, wv_d),
                              (bqk_sb, bqk_d), (bv_sb, bv_d),
                              (ones_sb, ones_d)]:
                nc.sync.dma_start(sb_t[:], d_t[:])

            sgb = pp.tile([128, KT, RS], dt.bfloat16)      # [j%128, jt, i]
            aj_all = pp.tile([128, KT, D], dt.bfloat16)    # [j%128, jt, r*64+hd]
            T1 = pp.tile([128, N], dt.bfloat16)    # [qT_lo ; kT_hi]
            T2 = pp.tile([128, N], dt.bfloat16)    # [kT_lo ; qT_hi]
            vh = pp.tile([128, KT, 128], dt.bfloat16)  # [key, kt, hd|1|0pad]
            attnT_sb = pp.tile([HD, N], dt.bfloat16)   # [hd, seq]
            scr = pp.tile([1, 16], dt.float32)

            bounce = [dp.tile([HD, c1 - c0], dt.bfloat16, name=f"bounce{i}")
                      for i, (c0, c1, _) in enumerate(AG_CHUNKS)]
            gath = [dp.tile([NC * HD, c1 - c0], dt.bfloat16,
                            addr_space="Shared", name=f"gath{i}")
                    for i, (c0, c1, _) in enumerate(AG_CHUNKS)]

            with tc.tile_pool(name="inbuf", bufs=1) as ib:
                kT_sb = ib.tile([128, QB, NB, 512], dt.float8e4)
                qT_sb = ib.tile([128, QB, NB, 512], dt.float8e4)
                vT_sb = ib.tile([128, QB, NB, 512], dt.bfloat16)

                # input prefetch: few fat kicks (each DMA kick costs ~0.65us
                # of sync-engine issue time, so 24 small chunks starved the
                # first matmul for ~19us)
                for lo, hi in [(0, 4), (4, 8)]:
                    nc.sync.dma_start(kT_sb[:, lo:hi], kT_d[:, lo:hi])
                    nc.sync.dma_start(qT_sb[:, lo:hi], qT_d[:, lo:hi])
                    nc.sync.dma_start(vT_sb[:, lo:hi], vT_d[:, lo:hi])
                # final-projection weights: needed last, loaded last
                nc.sync.dma_start(wo_sb[:], wo_d[:])
                nc.sync.dma_start(bo_sb[:], bo_d[:])
                nc.vector.memset(vh[:, :, HD:128], 0.0)
                nc.vector.memset(vh[:, :, HD:HD + 1], 1.0)
                # preload the exp table set on ACT while DMAs run
                nc.scalar.activation(scr[:, 0:1], bqk_sb[0:1, :], Exp)

                # -------- Phase A: q/k/v projections, chunk-pipelined -------
                with tc.tile_pool(name="pa_ps", bufs=2, space="PSUM") as pa_ps, \
                     tc.tile_pool(name="pv_ps", bufs=3, space="PSUM") as pv_ps:
                    def qk_block(nb):
                        sl = slice(nb * 512, (nb + 1) * 512)
                        ps = pa_ps.tile([128, 512], dt.float32, tag="pa")
                        for kb in range(NB):
                            nc.tensor.matmul(ps[0:64, :], wq_sb[:, kb, :],
                                             qT_sb[:, nb, kb, :],
                                             start=(kb == 0),
                                             stop=(kb == NB - 1),
                                             tile_position=(0, 0))
                            nc.tensor.matmul(ps[64:128, :], wk_sb[:, kb, :],
                                             kT_sb[:, nb, kb, :],
                                             start=(kb == 0),
                                             stop=(kb == NB - 1),
                                             tile_position=(0, 64),
                                             skip_group_check=True)
                        nc.scalar.activation(T1[:, sl], ps[:], Ident,
                                             bias=bqk_sb[:],
                                             scale=1.0 / 64.0)
                        nc.sync.dma_start(T2[0:64, sl], T1[64:128, sl])
                        nc.sync.dma_start(T2[64:128, sl], T1[0:64, sl])

                    def v_block(nb):
                        for t in range(4):
                            nt = nb * 4 + t
                            psv = pv_ps.tile([128, HD], dt.float32, tag="pv")
                            for kb in range(NB):
                                nc.tensor.matmul(
                                    psv[:],
                                    vT_sb[:, nb, kb, t * 128:(t + 1) * 128],
                                    wv_sb[:, kb, :],
                                    start=(kb == 0), stop=False)
                            nc.tensor.matmul(psv[:], ones_sb[0:1, :], bv_sb[:],
                                             start=False, stop=True)
                            nc.vector.tensor_copy(vh[:, nt, 0:HD], psv[:])

                    for nb in range(4):
                        qk_block(nb)
                    for nb in range(4):
                        v_block(nb)
                    for nb in range(4, QB):
                        qk_block(nb)
                    for nb in range(4, QB):
                        v_block(nb)
                # sgconv matrix load: queued behind the T2 swaps so it never
                # delays them; done by ~31us, needed only from ~110us
                for c in range(4):
                    nc.sync.dma_start(sgb[:, c * 8:(c + 1) * 8, :],
                                      sgT_d[:, c * 8:(c + 1) * 8, :])

                # ---------------- Phase B: attention ----------------
                with tc.tile_pool(name="s_ps", bufs=3, space="PSUM") as s_pool, \
                     tc.tile_pool(name="o_ps", bufs=2, space="PSUM") as o_pool, \
                     tc.tile_pool(name="pt", bufs=3) as pt_pool, \
                     tc.tile_pool(name="os", bufs=2) as os_pool, \
                     tc.tile_pool(name="rc", bufs=2) as rc_pool:

                    def emit_S(qsl, g):
                        ktA, ktB = 2 * g, 2 * g + 1
                        sps = s_pool.tile([128, 1024], dt.float32, tag="sps")
                        nc.tensor.matmul(
                            sps[:, 0:512], T2[0:64, ktA * 128:(ktA + 1) * 128],
                            T1[0:64, qsl], start=True, stop=True,
                            tile_position=(0, 0))
                        nc.tensor.matmul(
                            sps[:, 512:1024],
                            T1[64:128, ktB * 128:(ktB + 1) * 128],
                            T2[64:128, qsl], start=True, stop=True,
                            tile_position=(64, 0), skip_group_check=True)
                        return sps

                    CHUNKS = [[0, 1, 2, 3], [4, 5, 6, 7],
                              [8, 9, 10, 11], [12, 13, 14, 15]]
                    tail = [None]   # deferred (qb, oc) for dbb/recip/mult

                    def emit_tail():
                        if tail[0] is None:
                            return
                        qb, oc = tail[0]
                        tail[0] = None
                        qsl = slice(qb * 512, (qb + 1) * 512)
                        # replicate the denominator row across partitions
                        # 0-63 with a K=1 matmul (ones at array row 64)
                        dbb = o_pool.tile([128, 512], dt.float32, tag="o")
                        nc.tensor.matmul(dbb[0:HD, :], ones_sb[HD:HD + 1, 0:HD],
                                         oc[HD:HD + 1, :], start=True,
                                         stop=True, tile_position=(64, 0),
                                         skip_group_check=True)
                        rc = rc_pool.tile([HD, 512], dt.float32, tag="rc")
                        nc.vector.reciprocal_approx_fast(rc[:], dbb[0:HD, :])
                        nc.vector.tensor_tensor(
                            attnT_sb[:, qsl], oc[0:HD, :], rc[:], Mult)

                    def emit_trs(ci):
                        c0, c1, _ = AG_CHUNKS[ci]
                        for j in range((c1 - c0) // 128):
                            jt = c0 // 128 + j
                            nc.sync.dma_start_transpose(
                                aj_all[:, jt, :],
                                gath[ci][:, j * 128:(j + 1) * 128])

                    def emit_ag(ci):
                        c0, c1, _ = AG_CHUNKS[ci]
                        nc.sync.dma_start(bounce[ci][:], attnT_sb[:, c0:c1])
                        nc.gpsimd.collective_compute(
                            "AllGather", mybir.AluOpType.bypass,
                            replica_groups=[list(range(NC))],
                            ins=[bounce[ci][:].opt()], outs=[gath[ci][:].opt()])
                        # transposes of the PREVIOUS chunk: their kicks
                        # inline-wait that AG, so they must sit behind this
                        # chunk's bounce/trigger, never ahead of it
                        if ci > 0:
                            emit_trs(ci - 1)

                    ag_at = {qb: ci for ci, (_, _, qb) in enumerate(AG_CHUNKS)
                             if qb is not None}
                    for qb in range(QB):
                        qsl = slice(qb * 512, (qb + 1) * 512)
                        o_ps = o_pool.tile([128, 512], dt.float32, tag="o")
                        store = {g: emit_S(qsl, g) for g in CHUNKS[0]}
                        emit_tail()
                        if qb in ag_at:
                            emit_ag(ag_at[qb])
                        for ci, ch in enumerate(CHUNKS):
                            ps_list = []
                            for g in ch:
                                sps = store.pop(g)
                                p = pt_pool.tile([128, 1024], dt.bfloat16,
                                                 tag="pt")
                                # split each pair across both engines: ACT
                                # takes the ktA half (exact exp), DVE the ktB
                                # half (Schraudolph) — halves per-pair latency
                                nc.scalar.activation(p[:, 0:512],
                                                     sps[:, 0:512], Exp)
                                nc.vector.tensor_scalar(
                                    p[:, 512:1024].bitcast(dt.int16),
                                    sps[:, 512:1024],
                                    FEXP_A, FEXP_B, Mult, Add)
                                ps_list.append((g, p))
                            if ci + 1 < len(CHUNKS):
                                for g in CHUNKS[ci + 1]:
                                    store[g] = emit_S(qsl, g)
                            for g, p in ps_list:
                                for t in range(2):
                                    kt = 2 * g + t
                                    nc.tensor.matmul(
                                        o_ps[:], vh[:, kt, :],
                                        p[:, t * 512:(t + 1) * 512],
                                        start=(g == 0 and t == 0),
                                        stop=(g == NPAIR - 1 and t == 1),
                                        skip_group_check=True)
                        # free o_ps fast; the rest of the normalize rides
                        # behind the next block's first S-run
                        oc = os_pool.tile([HD + 1, 512], dt.bfloat16, tag="oc")
                        nc.vector.tensor_copy(oc[:], o_ps[0:HD + 1, :])
                        tail[0] = (qb, oc)
                    emit_tail()
                    emit_ag(3)
                    emit_trs(3)

            # ---------------- Phase C: sgconv (out_sg^T) ----------------
            # aj_all[:, jt] tiles for jt 0-27 are ready before attention
            # ends; jt 28-31 arrive via the last AG, hidden behind the
            # first 112 sgconv matmuls.
            with tc.tile_pool(name="og_ps", bufs=1, space="PSUM") as og_pool, \
                 tc.tile_pool(name="pd_sb", bufs=1) as pd_sb_pool:
                og = og_pool.tile([128, NB, RS], dt.float32)
                for jt in range(KT):
                    for db in range(NB):
                        nc.tensor.matmul(
                            og[:, db, :],
                            aj_all[:, jt, db * 128:(db + 1) * 128],
                            sgb[:, jt, :], start=(jt == 0),
                            stop=(jt == KT - 1), skip_group_check=True)
                # ---------------- Phase D: final projection ----------------
                ogT = pd_sb_pool.tile([128, NB, RS], dt.bfloat16)
                for db in range(NB):
                    nc.vector.tensor_copy(ogT[:, db, :], og[:, db, :])
                with tc.tile_pool(name="pd_ps", bufs=2, space="PSUM") as pd_ps_pool, \
                     tc.tile_pool(name="po_sb", bufs=2) as po_sb_pool:
                    for it in range(NB):
                        ps = pd_ps_pool.tile([128, D], dt.float32, tag="pd")
                        for db in range(NB):
                            nc.tensor.matmul(
                                ps[:], ogT[:, db, it * 128:(it + 1) * 128],
                                wo_sb[:, db, :], start=(db == 0), stop=False)
                        nc.tensor.matmul(ps[:], ones_sb[0:1, :], bo_sb[:],
                                         start=False, stop=True)
                        po = po_sb_pool.tile([128, D], dt.float32, tag="po")
                        nc.vector.tensor_copy(po[:], ps[:])
                        nc.sync.dma_start(out_d[it * 128:(it + 1) * 128, :], po[:])
    nc.compile()
    return nc


def kernel(**inputs):
    query = np.asarray(inputs["query"], dtype=np.float32)
    key = np.asarray(inputs["key"], dtype=np.float32)
    value = np.asarray(inputs["value"], dtype=np.float32)
    Wq, bq = np.asarray(inputs["Wq"], np.float32), np.asarray(inputs["bq"], np.float32)
    Wk, bk = np.asarray(inputs["Wk"], np.float32), np.asarray(inputs["bk"], np.float32)
    Wv, bv = np.asarray(inputs["Wv"], np.float32), np.asarray(inputs["bv"], np.float32)
    Wo, bo = np.asarray(inputs["Wo"], np.float32), np.asarray(inputs["bo"], np.float32)
    sg = np.asarray(inputs["sgconv_mat"], np.float32)[0]   # [N, N]

    if "nc" not in _CACHE:
        _CACHE["nc"] = _build()
    nc = _CACHE["nc"]

    qTb = blkq(query[0].T.astype(fp8))
    kTb = blkq(key[0].T.astype(fp8))
    vTb = blkq(value[0].T.astype(bf16))
    wob = blk(Wo.T.astype(bf16))
    common = {
        "qT": qTb, "kT": kTb, "vT": vTb, "wo": wob,
        "bo": bo.reshape(1, D).astype(bf16),
        "ones": np.ones((128, 128), bf16),
        "eye": np.eye(128, dtype=np.float32),
    }
    in_maps = []
    for c in range(NC):
        hs = slice(c * HD, (c + 1) * HD)
        rs = slice(c * RS, (c + 1) * RS)
        sgT = np.ascontiguousarray(
            sg[rs, :].T.reshape(KT, 128, RS).transpose(1, 0, 2)).astype(bf16)
        in_maps.append(dict(
            common,
            wq=blk((64.0 * SCALE * Wq[hs, :]).T.astype(fp8)),
            wk=blk((64.0 * Wk[hs, :]).T.astype(fp8)),
            wv=blk(Wv[hs, :].T.astype(bf16)),
            bqk=np.concatenate([SCALE * bq[hs], bk[hs]]).reshape(128, 1)
                .astype(np.float32),
            bv=bv[hs].reshape(1, HD).astype(bf16),
            sgT=sgT,
        ))
    res = run_bass_kernel_spmd(nc, in_maps, core_ids=list(range(NC)),
                               **_CACHE.get("run_kwargs", {}))
    _CACHE["last_results"] = res
    out = np.concatenate([res.results[c]["out"] for c in range(NC)], axis=0)
    return out.reshape(1, N, D)


# revision 65
# speedup vs baseline: 1.3272x; 1.3272x over previous
"""GraphWeightedMHA on 8 trn2 cores — head-sharded bf16 Bass/Tile kernel.

Sharding: one attention head per core (tensor parallel); sgconv + final
projection row-sharded. Only the per-head [4096, 64] attention output is
AllGathered (4 chunks issued at qb2/qb4/qb7/end so the collectives and
gather-side transposes overlap attention).

Key scheduling facts this kernel is built around (from HW traces):
- Every DMA kick costs ~0.65us (plain) / ~1.3us (XBAR transpose) of
  in-order sync-engine issue time, and a kick inline-blocks the engine on
  its wait semaphore. So: few fat input DMAs (re-blocked host layouts with
  2-16KB contiguous partition lines), and each AG chunk's gather-side
  transposes are emitted one chunk LATE — behind the next chunk's bounce
  DMA + collective trigger — so the AG chain never serializes on
  transpose kicks waiting for the previous AG.
- DMA kicks must stay off compute engines (scalar-queue kicks interleave
  guard waits with ACT work and can near-deadlock).
- exp split 9 ACT / 7 DVE pairs (ACT 1.10us, DVE 1.22us per [128,1024]
  fp32->bf16 tile; fp32 PSUM reads cap DVE at 1 elem/cycle).
- sgconv runs jt 0..31 in order right behind the last attention block;
  its first ~24us of matmuls hide the final AG + transposes.

All matmuls bf16 (fp32 PSUM accumulation); q/k inputs fp8.
"""
import numpy as np
import ml_dtypes
import concourse.bass as bass
import concourse.bacc as bacc
import concourse.tile as tile
from concourse import mybir
from concourse.bass_utils import run_bass_kernel_spmd

dt = mybir.dt
bf16 = ml_dtypes.bfloat16
fp8 = ml_dtypes.float8_e4m3fn
NC = 8
N, D, H, HD = 4096, 512, 8, 64
RS = N // NC          # 512 output rows per core for sgconv/final proj
NB = D // 128         # 4 blocks of 128 along D
QB = 8                # query blocks of 512
KT = N // 128         # 32 key tiles
NPAIR = KT // 2       # 16 S^T pairs per query block
SCALE = float(1.0 / np.sqrt(np.float32(D)))
Exp = mybir.ActivationFunctionType.Exp
Ident = mybir.ActivationFunctionType.Identity
Mult = mybir.AluOpType.mult
Add = mybir.AluOpType.add

# Schraudolph fast-exp in bf16 bit space: bf16_bits(exp(x)) ~ round(A*x + B)
FEXP_A = float(np.float32(128.0 / np.log(2.0)))
FEXP_B = float(np.float32(127 * 128 - 5.0))
# pairs using the scalar engine's exact exp (rest: DVE Schraudolph fast-exp)
ACT_SET = {0, 2, 4, 6, 8, 10, 12, 14, 15}

# AllGather chunks: (row0, row1, issue_at_qb) — issue_at_qb None = post-loop
AG_CHUNKS = [(0, 1024, 2), (1024, 2048, 4), (2048, 3584, 7), (3584, 4096, None)]

_CACHE: dict = {}


def blk(x):  # [512, M] -> [128, 4, M]  (p, kb, m) with d = kb*128+p
    return np.ascontiguousarray(x.reshape(NB, 128, -1).transpose(1, 0, 2))


def blkq(x):  # [512, N] -> [128, QB, NB, 512]: column-chunk-major for fat DMA
    return np.ascontiguousarray(
        blk(x).reshape(128, NB, QB, 512).transpose(0, 2, 1, 3))


def _build():
    nc = bacc.Bacc("TRN2", target_bir_lowering=False, debug=False, num_devices=NC)

    def din(name, shape, d=dt.bfloat16):
        return nc.dram_tensor(name, shape, d, kind="ExternalInput").ap()

    qT_d = din("qT", [128, QB, NB, 512], dt.float8e4)  # query^T re-blocked (fp8)
    kT_d = din("kT", [128, QB, NB, 512], dt.float8e4)  # key^T re-blocked (fp8)
    vT_d = din("vT", [128, QB, NB, 512])       # value^T re-blocked (bf16)
    wq_d = din("wq", [128, NB, HD], dt.float8e4)  # (64*s*Wq_h)^T blocked
    wk_d = din("wk", [128, NB, HD], dt.float8e4)  # (64*Wk_h)^T blocked
    wv_d = din("wv", [128, NB, HD])            # Wv_h^T blocked (bf16)
    wo_d = din("wo", [128, NB, D])             # Wo^T blocked
    bqk_d = din("bqk", [128, 1], dt.float32)   # [s*bq_h ; bk_h]
    bv_d = din("bv", [1, HD])                  # bv_h row (bf16)
    bo_d = din("bo", [1, D])
    ones_d = din("ones", [128, 128])
    sgT_d = din("sgT", [128, KT, RS])          # sg[rows,:].T pre-blocked, bf16
    out_d = nc.dram_tensor("out", [RS, D], dt.float32, kind="ExternalOutput").ap()

    with tile.TileContext(nc) as tc:
        with tc.tile_pool(name="const", bufs=1) as cp, \
             tc.tile_pool(name="persist", bufs=1) as pp, \
             tc.tile_pool(name="dram", bufs=1, space="DRAM") as dp:
            wq_sb = cp.tile([128, NB, HD], dt.float8e4)
            wk_sb = cp.tile([128, NB, HD], dt.float8e4)
            wv_sb = cp.tile([128, NB, HD], dt.bfloat16)
            wo_sb = cp.tile([128, NB, D], dt.bfloat16)
            bqk_sb = cp.tile([128, 1], dt.float32)
            bv_sb = cp.tile([1, HD], dt.bfloat16)
            bo_sb = cp.tile([1, D], dt.bfloat16)
            ones_sb = cp.tile([128, 128], dt.bfloat16)
            # critical-path weights first; wo/bo (final proj only) last
            for sb_t, d_t in [(wq_sb, wq_d), (wk_sb, wk_d), (wv_sb, wv_d),
                              (bqk_sb, bqk_d), (bv_sb, bv_d),
                              (ones_sb, ones_d)]:
                nc.sync.dma_start(sb_t[:], d_t[:])

            sgb = pp.tile([128, KT, RS], dt.bfloat16)      # [j%128, jt, i]
            aj_all = pp.tile([128, KT, D], dt.bfloat16)    # [j%128, jt, r*64+hd]
            T1 = pp.tile([128, N], dt.bfloat16)    # [qT_lo ; kT_hi]
            T2 = pp.tile([128, N], dt.bfloat16)    # [kT_lo ; qT_hi]
            vh = pp.tile([128, KT, 128], dt.bfloat16)  # [key, kt, hd|1|0pad]
            attnT_sb = pp.tile([HD, N], dt.bfloat16)   # [hd, seq]
            scr = pp.tile([1, 16], dt.float32)

            bounce = [dp.tile([HD, c1 - c0], dt.bfloat16, name=f"bounce{i}")
                      for i, (c0, c1, _) in enumerate(AG_CHUNKS)]
            gath = [dp.tile([NC * HD, c1 - c0], dt.bfloat16,
                            addr_space="Shared", name=f"gath{i}")
                    for i, (c0, c1, _) in enumerate(AG_CHUNKS)]

            with tc.tile_pool(name="inbuf", bufs=1) as ib:
                kT_sb = ib.tile([128, QB, NB, 512], dt.float8e4)
                qT_sb = ib.tile([128, QB, NB, 512], dt.float8e4)
                vT_sb = ib.tile([128, QB, NB, 512], dt.bfloat16)

                # input prefetch: few fat kicks (each DMA kick costs ~0.65us
                # of sync-engine issue time, so 24 small chunks starved the
                # first matmul for ~19us)
                for lo, hi in [(0, 4), (4, 8)]:
                    nc.sync.dma_start(kT_sb[:, lo:hi], kT_d[:, lo:hi])
                    nc.sync.dma_start(qT_sb[:, lo:hi], qT_d[:, lo:hi])
                    nc.sync.dma_start(vT_sb[:, lo:hi], vT_d[:, lo:hi])
                # final-projection weights: needed last, loaded last
                nc.sync.dma_start(wo_sb[:], wo_d[:])
                nc.sync.dma_start(bo_sb[:], bo_d[:])
                nc.vector.memset(vh[:, :, HD:128], 0.0)
                nc.vector.memset(vh[:, :, HD:HD + 1], 1.0)
                # preload the exp table set on ACT while DMAs run
                nc.scalar.activation(scr[:, 0:1], bqk_sb[0:1, :], Exp)

                # -------- Phase A: q/k/v projections, chunk-pipelined -------
                with tc.tile_pool(name="pa_ps", bufs=2, space="PSUM") as pa_ps, \
                     tc.tile_pool(name="pv_ps", bufs=3, space="PSUM") as pv_ps:
                    def qk_block(nb):
                        sl = slice(nb * 512, (nb + 1) * 512)
                        ps = pa_ps.tile([128, 512], dt.float32, tag="pa")
                        for kb in range(NB):
                            nc.tensor.matmul(ps[0:64, :], wq_sb[:, kb, :],
                                             qT_sb[:, nb, kb, :],
                                             start=(kb == 0),
                                             stop=(kb == NB - 1),
                                             tile_position=(0, 0))
                            nc.tensor.matmul(ps[64:128, :], wk_sb[:, kb, :],
                                             kT_sb[:, nb, kb, :],
                                             start=(kb == 0),
                                             stop=(kb == NB - 1),
                                             tile_position=(0, 64),
                                             skip_group_check=True)
                        nc.scalar.activation(T1[:, sl], ps[:], Ident,
                                             bias=bqk_sb[:],
                                             scale=1.0 / 64.0)
                        nc.sync.dma_start(T2[0:64, sl], T1[64:128, sl])
                        nc.sync.dma_start(T2[64:128, sl], T1[0:64, sl])

                    def v_block(nb):
                        for t in range(4):
                            nt = nb * 4 + t
                            psv = pv_ps.tile([128, HD], dt.float32, tag="pv")
                            for kb in range(NB):
                                nc.tensor.matmul(
                                    psv[:],
                                    vT_sb[:, nb, kb, t * 128:(t + 1) * 128],
                                    wv_sb[:, kb, :],
                                    start=(kb == 0), stop=False)
                            nc.tensor.matmul(psv[:], ones_sb[0:1, :], bv_sb[:],
                                             start=False, stop=True)
                            nc.vector.tensor_copy(vh[:, nt, 0:HD], psv[:])

                    for nb in range(4):
                        qk_block(nb)
                    for nb in range(4):
                        v_block(nb)
                    for nb in range(4, QB):
                        qk_block(nb)
                    for nb in range(4, QB):
                        v_block(nb)
                # sgconv matrix load: queued behind the T2 swaps so it never
                # delays them; done by ~31us, needed only from ~110us
                for c in range(4):
                    nc.sync.dma_start(sgb[:, c * 8:(c + 1) * 8, :],
                                      sgT_d[:, c * 8:(c + 1) * 8, :])

                # ---------------- Phase B: attention ----------------
                with tc.tile_pool(name="s_ps", bufs=3, space="PSUM") as s_pool, \
                     tc.tile_pool(name="o_ps", bufs=2, space="PSUM") as o_pool, \
                     tc.tile_pool(name="pt", bufs=3) as pt_pool, \
                     tc.tile_pool(name="os", bufs=2) as os_pool, \
                     tc.tile_pool(name="rc", bufs=2) as rc_pool:

                    def emit_S(qsl, g):
                        ktA, ktB = 2 * g, 2 * g + 1
                        sps = s_pool.tile([128, 1024], dt.float32, tag="sps")
                        nc.tensor.matmul(
                            sps[:, 0:512], T2[0:64, ktA * 128:(ktA + 1) * 128],
                            T1[0:64, qsl], start=True, stop=True,
                            tile_position=(0, 0))
                        nc.tensor.matmul(
                            sps[:, 512:1024],
                            T1[64:128, ktB * 128:(ktB + 1) * 128],
                            T2[64:128, qsl], start=True, stop=True,
                            tile_position=(64, 0), skip_group_check=True)
                        return sps

                    CHUNKS = [[0, 1, 2, 3], [4, 5, 6, 7],
                              [8, 9, 10, 11], [12, 13, 14, 15]]
                    tail = [None]   # deferred (qb, oc) for dbb/recip/mult

                    def emit_tail():
                        if tail[0] is None:
                            return
                        qb, oc = tail[0]
                        tail[0] = None
                        qsl = slice(qb * 512, (qb + 1) * 512)
                        # replicate the denominator row across partitions
                        # 0-63 with a K=1 matmul (ones at array row 64)
                        dbb = o_pool.tile([128, 512], dt.float32, tag="o")
                        nc.tensor.matmul(dbb[0:HD, :], ones_sb[HD:HD + 1, 0:HD],
                                         oc[HD:HD + 1, :], start=True,
                                         stop=True, tile_position=(64, 0),
                                         skip_group_check=True)
                        rc = rc_pool.tile([HD, 512], dt.float32, tag="rc")
                        nc.vector.reciprocal_approx_fast(rc[:], dbb[0:HD, :])
                        nc.vector.tensor_tensor(
                            attnT_sb[:, qsl], oc[0:HD, :], rc[:], Mult)

                    def emit_trs(ci):
                        c0, c1, _ = AG_CHUNKS[ci]
                        for j in range((c1 - c0) // 128):
                            jt = c0 // 128 + j
                            nc.sync.dma_start_transpose(
                                aj_all[:, jt, :],
                                gath[ci][:, j * 128:(j + 1) * 128])

                    def emit_ag(ci):
                        c0, c1, _ = AG_CHUNKS[ci]
                        nc.sync.dma_start(bounce[ci][:], attnT_sb[:, c0:c1])
                        nc.gpsimd.collective_compute(
                            "AllGather", mybir.AluOpType.bypass,
                            replica_groups=[list(range(NC))],
                            ins=[bounce[ci][:].opt()], outs=[gath[ci][:].opt()])
                        # transposes of the PREVIOUS chunk: their kicks
                        # inline-wait that AG, so they must sit behind this
                        # chunk's bounce/trigger, never ahead of it
                        if ci > 0:
                            emit_trs(ci - 1)

                    ag_at = {qb: ci for ci, (_, _, qb) in enumerate(AG_CHUNKS)
                             if qb is not None}
                    for qb in range(QB):
                        qsl = slice(qb * 512, (qb + 1) * 512)
                        o_ps = o_pool.tile([128, 512], dt.float32, tag="o")
                        store = {g: emit_S(qsl, g) for g in CHUNKS[0]}
                        emit_tail()
                        if qb in ag_at:
                            emit_ag(ag_at[qb])
                        for ci, ch in enumerate(CHUNKS):
                            ps_list = []
                            for g in ch:
                                sps = store.pop(g)
                                p = pt_pool.tile([128, 1024], dt.bfloat16,
                                                 tag="pt")
                                if g in ACT_SET:
                                    nc.scalar.activation(p[:], sps[:], Exp)
                                else:
                                    nc.vector.tensor_scalar(
                                        p[:].bitcast(dt.int16), sps[:],
                                        FEXP_A, FEXP_B, Mult, Add)
                                ps_list.append((g, p))
                            if ci + 1 < len(CHUNKS):
                                for g in CHUNKS[ci + 1]:
                                    store[g] = emit_S(qsl, g)
                            for g, p in ps_list:
                                for t in range(2):
                                    kt = 2 * g + t
                                    nc.tensor.matmul(
                                        o_ps[:], vh[:, kt, :],
                                        p[:, t * 512:(t + 1) * 512],
                                        start=(g == 0 and t == 0),
                                        stop=(g == NPAIR - 1 and t == 1),
                                        skip_group_check=True)
                        # free o_ps fast; the rest of the normalize rides
                        # behind the next block's first S-run
                        oc = os_pool.tile([HD + 1, 512], dt.bfloat16, tag="oc")
                        nc.vector.tensor_copy(oc[:], o_ps[0:HD + 1, :])
                        tail[0] = (qb, oc)
                    emit_tail()
                    emit_ag(3)
                    emit_trs(3)

            # ---------------- Phase C: sgconv (out_sg^T) ----------------
            # aj_all[:, jt] tiles for jt 0-27 are ready before attention
            # ends; jt 28-31 arrive via the last AG, hidden behind the
            # first 112 sgconv matmuls.
            with tc.tile_pool(name="og_ps", bufs=1, space="PSUM") as og_pool, \
                 tc.tile_pool(name="pd_sb", bufs=1) as pd_sb_pool:
                og = og_pool.tile([128, NB, RS], dt.float32)
                for jt in range(KT):
                    for db in range(NB):
                        nc.tensor.matmul(
                            og[:, db, :],
                            aj_all[:, jt, db * 128:(db + 1) * 128],
                            sgb[:, jt, :], start=(jt == 0),
                            stop=(jt == KT - 1), skip_group_check=True)
                # ---------------- Phase D: final projection ----------------
                ogT = pd_sb_pool.tile([128, NB, RS], dt.bfloat16)
                for db in range(NB):
                    nc.vector.tensor_copy(ogT[:, db, :], og[:, db, :])
                with tc.tile_pool(name="pd_ps", bufs=2, space="PSUM") as pd_ps_pool, \
                     tc.tile_pool(name="po_sb", bufs=2) as po_sb_pool:
                    for it in range(NB):
                        ps = pd_ps_pool.tile([128, D], dt.float32, tag="pd")
                        for db in range(NB):
                            nc.tensor.matmul(
                                ps[:], ogT[:, db, it * 128:(it + 1) * 128],
                                wo_sb[:, db, :], start=(db == 0), stop=False)
                        nc.tensor.matmul(ps[:], ones_sb[0:1, :], bo_sb[:],
                                         start=False, stop=True)
                        po = po_sb_pool.tile([128, D], dt.float32, tag="po")
                        nc.vector.tensor_copy(po[:], ps[:])
                        nc.sync.dma_start(out_d[it * 128:(it + 1) * 128, :], po[:])
    nc.compile()
    return nc


def kernel(**inputs):
    query = np.asarray(inputs["query"], dtype=np.float32)
    key = np.asarray(inputs["key"], dtype=np.float32)
    value = np.asarray(inputs["value"], dtype=np.float32)
    Wq, bq = np.asarray(inputs["Wq"], np.float32), np.asarray(inputs["bq"], np.float32)
    Wk, bk = np.asarray(inputs["Wk"], np.float32), np.asarray(inputs["bk"], np.float32)
    Wv, bv = np.asarray(inputs["Wv"], np.float32), np.asarray(inputs["bv"], np.float32)
    Wo, bo = np.asarray(inputs["Wo"], np.float32), np.asarray(inputs["bo"], np.float32)
    sg = np.asarray(inputs["sgconv_mat"], np.float32)[0]   # [N, N]

    if "nc" not in _CACHE:
        _CACHE["nc"] = _build()
    nc = _CACHE["nc"]

    qTb = blkq(query[0].T.astype(fp8))
    kTb = blkq(key[0].T.astype(fp8))
    vTb = blkq(value[0].T.astype(bf16))
    wob = blk(Wo.T.astype(bf16))
    common = {
        "qT": qTb, "kT": kTb, "vT": vTb, "wo": wob,
        "bo": bo.reshape(1, D).astype(bf16),
        "ones": np.ones((128, 128), bf16),
        "eye": np.eye(128, dtype=np.float32),
    }
    in_maps = []
    for c in range(NC):
        hs = slice(c * HD, (c + 1) * HD)
        rs = slice(c * RS, (c + 1) * RS)
        sgT = np.ascontiguousarray(
            sg[rs, :].T.reshape(KT, 128, RS).transpose(1, 0, 2)).astype(bf16)
        in_maps.append(dict(
            common,
            wq=blk((64.0 * SCALE * Wq[hs, :]).T.astype(fp8)),
            wk=blk((64.0 * Wk[hs, :]).T.astype(fp8)),
            wv=blk(Wv[hs, :].T.astype(bf16)),
            bqk=np.concatenate([SCALE * bq[hs], bk[hs]]).reshape(128, 1)
                .astype(np.float32),
            bv=bv[hs].reshape(1, HD).astype(bf16),
            sgT=sgT,
        ))
    res = run_bass_kernel_spmd(nc, in_maps, core_ids=list(range(NC)),
                               **_CACHE.get("run_kwargs", {}))
    _CACHE["last_results"] = res
    out = np.concatenate([res.results[c]["out"] for c in range(NC)], axis=0)
    return out.reshape(1, N, D)
